# revision 28
# baseline (speedup 1.0000x reference)
"""Multi-head attention TRN2 kernel (nn_MultiHeadAttention_69922067579127).

Full-input contract: kernel(**inputs) takes the complete tensors and
returns the complete output. Internally: tensor-parallel over heads --
each of the 8 NeuronCores computes 2 of the 16 heads (QKV projection,
attention, and its slice of the output projection); the 8 partial
outputs are summed on the host (the output projection is linear in the
per-head contributions) and b_out is added once.

v3 design (fp16, DMA-XBAR transposes, cross-sweep software pipeline):
  - x and the weights are cast to fp16 on the host. x^T tiles are
    loaded straight from DRAM with the DMA crossbar transpose
    (dma_start_transpose, 2-byte dtypes), so the PE does ZERO
    transpose-mode ops (transpose-mode doesn't count as PE-busy for the
    HAM clock gate, which kept earlier versions at 1.2 GHz).
  - QKV^T [384, tok] = W.T @ x^T in fp16 (FWL weight loads), fp32 PSUM,
    bias added on DVE with fp16 output.
  - scores^T [k_tok, q_tok]: per kt the two heads' K=64 matmuls are
    emitted back-to-back targeting PE row groups 0-63 / 64-127 so the
    hardware runs them concurrently.
  - softmax without max-subtraction (scores ~ N(0,1)); exp on ScalarE
    with the 1/8 scale folded in, fp16 output; per-head denominators
    come from ones-columns in the AV stationaries.
  - V is re-laid token-major via SBUF->SBUF DMA transposes (not PE).
    h0's AV stationary is [v(64) | 1 | 1] -> rows 0-65 of its PSUM;
    h1's is [1 | 1 | junk(62) | v(64)] (128 cols) so h1's attention
    lands directly on PSUM partitions 64-127 (denominator on rows 0-1),
    which removes the SBUF->SBUF partition-shift DMA entirely -- every
    normalization op is partition-aligned.
  - normalization: K=1 broadcast matmuls of the raw denominators +
    single-op DVE approx reciprocal (~18 bits), multiplied into attnT.
  - cross-sweep pipeline: each sweep's normalization + output
    projection is deferred into the FIRST kt steps of the next sweep
    (where the PE would otherwise wait on exp), and the next batch's
    QKV runs in the remaining kt-step slack -- the exp cadence on
    ScalarE paces the whole kernel, and the PE never idles long enough
    for the HAM clock gate to re-throttle.
"""

import sys

sys.path.insert(0, "/opt/trn_rl_repo")

from contextlib import ExitStack

import numpy as np

import concourse.bacc as bacc
import concourse.mybir as mybir
import concourse.tile as tile
from concourse.bass_utils import run_bass_kernel_spmd

F16 = mybir.dt.float16
F32 = mybir.dt.float32
F32R = mybir.dt.float32r
EXP = mybir.ActivationFunctionType.Exp

B, T, D = 4, 2048, 1024
H, Dh = 16, 64
BT = B * T            # 8192 tokens
NCORES = 8
HPC = H // NCORES     # 2 heads per core
QC = 512              # query-sweep width (columns of S^T per block)
KT = T // 128         # 16 key-token tiles per batch
TC = 512              # token chunk for QKV
NTC = T // TC         # 4 per batch

_CACHE = {}


def _build():
    nc = bacc.Bacc("TRN2", target_bir_lowering=False, debug=False)
    x = nc.dram_tensor("x", [128, BT // TC, 8, TC], F16, kind="ExternalInput").ap()
    wqkv = nc.dram_tensor("wqkv", [D, 3 * 128], F16, kind="ExternalInput").ap()
    bqkv = nc.dram_tensor("bqkv", [3 * 128], F32, kind="ExternalInput").ap()
    wout = nc.dram_tensor("wout", [128, D], F16, kind="ExternalInput").ap()
    out = nc.dram_tensor("out", [BT, D], F32, kind="ExternalOutput").ap()

    with tile.TileContext(nc) as tc, ExitStack() as ctx:
        const = ctx.enter_context(tc.tile_pool(name="const", bufs=1))
        perb = ctx.enter_context(tc.tile_pool(name="perb", bufs=1))
        perb2 = ctx.enter_context(tc.tile_pool(name="perb2", bufs=3))
        xtp = ctx.enter_context(tc.tile_pool(name="xtp", bufs=2))
        stp = ctx.enter_context(tc.tile_pool(name="stp", bufs=7))
        work = ctx.enter_context(tc.tile_pool(name="work", bufs=2))
        outp = ctx.enter_context(tc.tile_pool(name="outp", bufs=3))
        # PSUM: 8 banks total. "mm" 2x1 + "sc" 2x2 + "av" 2x1 = 8.
        psA = ctx.enter_context(tc.tile_pool(name="psA", bufs=2, space="PSUM"))
        pssc = ctx.enter_context(tc.tile_pool(name="pssc", bufs=2, space="PSUM"))
        psav = ctx.enter_context(tc.tile_pool(name="psav", bufs=2, space="PSUM"))

        # ---- constants ----
        ones_f = const.tile([128, 128], F32)
        nc.vector.memset(ones_f, 1.0)
        ones_h = const.tile([128, 64], F16)
        nc.vector.tensor_copy(out=ones_h, in_=ones_f[:, 0:64])
        # [128, KT, 2] view for the per-kt ones pairs in vtok
        ones_k2 = ones_h[:, 0:32].rearrange("p (k c) -> p k c", c=2)

        ones_r = const.tile([128, 128], F32R)
        nc.vector.tensor_copy(out=ones_r, in_=ones_f)

        # K=65 broadcast selector: contraction rows 0..64; row 0 carries
        # 1/d1 (selected into output cols 64-127), row 64 carries d0
        # (cols 0-63), rows 1-63 are zero on both sides.
        sel_f = const.tile([128, 128], F32)
        nc.vector.memset(sel_f, 0.0)
        nc.vector.tensor_copy(out=sel_f[0:1, 64:128], in_=ones_f[0:1, 0:64])
        nc.vector.tensor_copy(out=sel_f[64:65, 0:64], in_=ones_f[64:65, 0:64])
        sel_r = const.tile([128, 128], F32R)
        nc.vector.tensor_copy(out=sel_r, in_=sel_f)

        zeros_f = const.tile([128, 1], F32)
        nc.vector.memset(zeros_f, 0.0)

        w_r = const.tile([128, 8, 384], F16)
        nc.sync.dma_start(out=w_r, in_=wqkv.rearrange("(ko ki) m -> ki ko m", ki=128))

        bq_sb = const.tile([128, 3], F32)
        nc.sync.dma_start(out=bq_sb, in_=bqkv.rearrange("(m p) -> p m", p=128))

        wo_r = const.tile([128, D], F16)
        nc.sync.dma_start(out=wo_r, in_=wout)

        # ---- persistent tiles ----
        vTt = perb.tile([128, T], F16)     # V^T (feature-major), per batch
        attnT = perb.tile([128, T], F16)   # normalized attn out (both heads)
        # rhs for the K=65 broadcast matmul: row 64 = d0, row 0 = 1/d1,
        # rows 1-63 zeroed once here and never touched again
        dtile = perb.tile([128, QC], F32R)
        nc.vector.tensor_scalar_mul(
            out=dtile[0:64, :], in0=dtile.bitcast(F32)[0:64, :], scalar1=zeros_f[0:64, :]
        )

        tiles = {}

        def start_b(bb):
            qT_b = perb2.tile([128, T], F16, tag="qT", name="qT")
            kT_b = perb2.tile([128, T], F16, tag="kT", name="kT")
            # token-major V per key-tile, 128-col stride per (kt, head):
            #   slot 0 (h0): [v(64) | 1 | 1 | unused(62)]   -> AV rows 0-65
            #   slot 1 (h1): [1 | 1 | junk(62) | v(64)]     -> AV rows 0-1
            #                 (denominator) and 64-127 (attention)
            # junk rows 2-63 of h1's AV output are never read.
            vtok_b = perb2.tile([128, KT, 2, 128], F16, tag="vtok", name="vtok")
            xt_b = xtp.tile([128, 8, T], F16, tag="xt", name="xt")
            nc.vector.tensor_copy(out=vtok_b[:, :, 0, 64:66], in_=ones_k2)
            nc.vector.tensor_copy(out=vtok_b[:, :, 1, 0:2], in_=ones_k2)
            # x^T for the whole batch via DMA crossbar transpose:
            # xt[p, ko, t] = x[bb*T + t, ko*128 + p]
            tiles[bb] = (qT_b, kT_b, vtok_b, xt_b)

        def emit_xt(bb, tci):
            # x arrives HOST-pre-transposed and pre-tiled as
            # [p, chunk, ko, t] so each partition reads one contiguous
            # 8KB block per chunk -- full HBM bandwidth, and nothing on
            # the serial XBAR transpose engine (which the vtok re-layout
            # still needs): xt[p, ko, t] = x[r0+t, ko*128 + p]
            xt_b = tiles[bb][3]
            nc.sync.dma_start(
                out=xt_b[:, :, tci * TC : (tci + 1) * TC],
                in_=x[:, bb * NTC + tci, :, :],
            )

        def phase_a_chunk(bb, tci):
            """Generator: QKV^T + token-major V for one 512-token chunk.
            Yields at ~1-matmul boundaries so the caller can interleave
            these PE ops into attention's exp-wait bubbles."""
            qT_b, kT_b, vtok_b, xt_b = tiles[bb]
            for m in range(3):
                psq = psA.tile([128, TC], F32, tag="mm", name="psq")
                for ko in range(8):
                    nc.tensor.matmul(
                        psq,
                        w_r[:, ko, m * 128 : (m + 1) * 128],
                        xt_b[:, ko, tci * TC : (tci + 1) * TC],
                        start=(ko == 0),
                        stop=(ko == 7),
                    )
                    yield
                dst = (qT_b, kT_b, vTt)[m]
                nc.vector.tensor_scalar_add(
                    out=dst[:, tci * TC : (tci + 1) * TC],
                    in0=psq,
                    scalar1=bq_sb[:, m : m + 1],
                )
            # V^T -> token-major V via SBUF->SBUF DMA transpose (one per
            # head): vtok[p, tci*4+j, h, .] = vTt[h*64+f, tci*512+j*128+p]
            nc.sync.dma_start_transpose(
                out=vtok_b[:, tci * 4 : (tci + 1) * 4, 0, 0:64],
                in_=vTt[0:64, tci * TC : (tci + 1) * TC],
            )
            nc.sync.dma_start_transpose(
                out=vtok_b[:, tci * 4 : (tci + 1) * 4, 1, 64:128],
                in_=vTt[64:128, tci * TC : (tci + 1) * TC],
            )
            yield

        def make_tail(bb, sw, av0, av1, emit_avs):
            """Generator: the finished sweep's five trailing AV pairs, its
            normalization, and its output projection. Consumed during the
            next sweep's first kt steps (after that sweep's scores, so the
            exp pipeline restarts with ~zero ScalarE gap)."""
            emit_avs(11, 12)
            yield
            emit_avs(13, 14)
            yield
            emit_avs(15)
            q0 = sw * QC
            # stage d0 (psum partition 64) and 1/d1 (via partition-0
            # reciprocal) into the persistent f32r broadcast-rhs tile
            nc.vector.tensor_copy(out=dtile[64:65, :], in_=av0[64:65, :])
            ddr = work.tile([1, QC], F32, tag="ddr", name="ddr")
            nc.vector.tensor_copy(out=ddr, in_=av1[0:1, :])
            rtmp = work.tile([1, QC], F32, tag="rtmp", name="rtmp")
            nc.vector.reciprocal_approx_fast(out=rtmp, in_=ddr)
            nc.vector.tensor_copy(out=dtile[0:1, :], in_=rtmp)
            yield
            # one K=65 matmul broadcasts [d0 | 1/d1] to rows [0-63 | 64-127]
            bcb = psA.tile([128, QC], F32, tag="mm", name="bcb")
            nc.tensor.matmul(
                bcb,
                sel_r[0:65, :],
                dtile[0:65, :],
                start=True,
                stop=True,
            )
            yield
            rc = work.tile([128, QC], F32, tag="rc", name="rc")
            nc.vector.reciprocal_approx_fast(out=rc[0:64, :], in_=bcb[0:64, :])
            nc.vector.tensor_copy(out=rc[64:128, :], in_=bcb[64:128, :])
            yield
            nc.vector.tensor_mul(
                out=attnT[0:64, q0 : q0 + QC], in0=av0[0:64, :], in1=rc[0:64, :]
            )
            nc.vector.tensor_mul(
                out=attnT[64:128, q0 : q0 + QC],
                in0=av1[64:128, :],
                in1=rc[64:128, :],
            )
            yield
            # output projection for this sweep's 4 q-slices (K=128 merged)
            for si in range(4):
                sl = sw * 4 + si
                outsb = outp.tile([128, D], F32, tag="outsb", name="outsb")
                for n in range(2):
                    po = psA.tile([128, 512], F32, tag="mm", name="po")
                    nc.tensor.matmul(
                        po,
                        attnT[:, sl * 128 : (sl + 1) * 128],
                        wo_r[:, n * 512 : (n + 1) * 512],
                        start=True,
                        stop=True,
                    )
                    nc.vector.tensor_copy(
                        out=outsb[:, n * 512 : (n + 1) * 512], in_=po
                    )
                r0 = bb * T + sl * 128
                nc.sync.dma_start(out=out[r0 : r0 + 128, :], in_=outsb)
                yield

        # per-kt-slot consumption counts: (tail steps, filler steps).
        # tail has 8 yield-groups: [av(14)+av(15)], [drow+bc], [rec],
        # [muls], [op]x4. filler (one QKV matmul per step) runs ONLY
        # after the tail is fully consumed: the tail's bc/po tiles and
        # the filler's psq accumulator share the 2-bank "mm" PSUM pool,
        # and allocating a bc/po tile while a psq accumulation group is
        # open would clear the bank mid-accumulation.
        # tail yield-groups (9): [av11,av12], [av13,av14],
        # [av15+denorm-stage], [K=2 bc matmul], [rec+copy], [muls],
        # [op]x4 -- consumed one per slot at kt0-9. Filler (one QKV
        # matmul per step) occupies kt2-5 with EXACTLY one 8-matmul psq
        # accumulation group (closing before the first po allocation at
        # kt6 -- the tail's bc/po tiles share the 2-bank "mm" PSUM pool
        # with psq, and an allocation inside an open group would clear
        # the bank mid-accumulation), then resumes at kt10.
        SLOT_PLAN = [
            (1, 0), (1, 0), (1, 2), (1, 2), (1, 2), (1, 2),
            (1, 0), (1, 0), (1, 0), (1, 0),
            (0, 3), (0, 3), (0, 3), (0, 3), (0, 3), (0, 2),
        ]

        def sweep(bb, sw, pre, filler):
            """One attention q-sweep (512 queries, both heads). `pre` is
            the previous sweep's tail; `filler` is next-batch phase A."""
            qT_b, kT_b, vtok_b, xt_b = tiles[bb]
            q0 = sw * QC
            av0 = psav.tile([66, QC], F32, tag="av", name="av0")
            av1 = psav.tile([128, QC], F32, tag="av", name="av1")
            sts = [None] * KT

            def _scores(kt):
                sc = pssc.tile([128, 1024], F32, tag="sc", name="sc")
                # two heads back-to-back on disjoint PE row groups ->
                # hardware runs them concurrently
                nc.tensor.matmul(
                    sc[:, 0:512],
                    kT_b[0:64, kt * 128 : (kt + 1) * 128],
                    qT_b[0:64, q0 : q0 + QC],
                    start=True,
                    stop=True,
                )
                nc.tensor.matmul(
                    sc[:, 512:1024],
                    kT_b[64:128, kt * 128 : (kt + 1) * 128],
                    qT_b[64:128, q0 : q0 + QC],
                    start=True,
                    stop=True,
                )
                st = stp.tile([128, 1024], F16, tag="st", name="st")
                nc.scalar.activation(out=st, in_=sc, func=EXP, scale=0.125)
                sts[kt] = st

            def _av(kt):
                st = sts[kt]
                nc.tensor.matmul(
                    av0,
                    vtok_b[:, kt, 0, 0:66],
                    st[:, 0:512],
                    start=(kt == 0),
                    stop=(kt == KT - 1),
                )
                nc.tensor.matmul(
                    av1,
                    vtok_b[:, kt, 1, 0:128],
                    st[:, 512:1024],
                    start=(kt == 0),
                    stop=(kt == KT - 1),
                )

            for kt in range(KT):
                _scores(kt)
                n_tail, n_fill = SLOT_PLAN[kt]
                for _ in range(n_tail):
                    next(pre, None)
                for _ in range(n_fill):
                    next(filler, None)
                if kt >= 5:
                    _av(kt - 5)
            for _ in pre:  # safety: tail must not outlive the sweep
                pass

            def emit_avs(*kts):
                for k in kts:
                    _av(k)

            return make_tail(bb, sw, av0, av1, emit_avs)

        # prologue: batch 0's phase A (and batch 1's chunk 3, so that
        # every batch's phase A finishes a full sweep before its own
        # sweeps begin -- the K/V bias-adds of a batch's last chunk
        # otherwise land exactly when the next batch's scores need them)
        start_b(0)
        start_b(1)
        for t in range(NTC):
            emit_xt(0, t)
        emit_xt(1, 3)
        emit_xt(1, 0)
        for t in range(NTC):
            for _ in phase_a_chunk(0, t):
                pass
        for _ in phase_a_chunk(1, 3):
            pass
        # phase-A chunk consumed during global sweep s (None = bare);
        # each batch's window ends one sweep before its own sweeps start
        CHUNK_OF_SWEEP = [
            (1, 0), (1, 1), (1, 2), (2, 3),
            (2, 0), (2, 1), (2, 2), (3, 3),
            (3, 0), (3, 1), (3, 2), None,
            None, None, None, None,
        ]
        # x^T chunk DMA-transposes, one sweep ahead of their chunk
        XT_OF_SWEEP = [
            (1, 1), (1, 2), (2, 3), (2, 0),
            (2, 1), (2, 2), (3, 3), (3, 0),
            (3, 1), (3, 2), None, None,
            None, None, None, None,
        ]
        pre = iter(())
        for b in range(B):
            for sw in range(4):
                s = b * 4 + sw
                if s == 2:
                    start_b(2)
                elif s == 6:
                    start_b(3)
                xe = XT_OF_SWEEP[s]
                if xe is not None:
                    emit_xt(*xe)
                ce = CHUNK_OF_SWEEP[s]
                filler = phase_a_chunk(*ce) if ce is not None else iter(())
                pre = sweep(b, sw, pre, filler)
                if _CACHE.get("inline_tail"):
                    for _ in pre:
                        pass
                    pre = iter(())
                for _ in filler:  # drain any leftover phase-A ops
                    pass
        for _ in pre:  # epilogue: drain the last sweep's tail
            pass

    nc.compile()
    return nc


def _in_maps(x, W_qkv, b_qkv, W_out):
    # pre-transpose + pre-tile x on the host into [p, chunk, ko, t]:
    # the device then loads x^T tiles with plain contiguous-read DMAs
    # instead of the serial XBAR transpose engine
    xf = np.ascontiguousarray(
        np.asarray(x, dtype=np.float32)
        .reshape(BT // TC, TC, 8, 128)
        .astype(np.float16)
        .transpose(3, 0, 2, 1)
    )
    W_qkv = np.asarray(W_qkv, dtype=np.float32)
    b_qkv = np.asarray(b_qkv, dtype=np.float32)
    W_out = np.asarray(W_out, dtype=np.float32)
    in_maps = []
    for c in range(NCORES):
        lo, hi = c * 128, (c + 1) * 128
        wq = np.ascontiguousarray(
            np.concatenate(
                [
                    W_qkv[:, lo:hi],
                    W_qkv[:, D + lo : D + hi],
                    W_qkv[:, 2 * D + lo : 2 * D + hi],
                ],
                axis=1,
            )
        ).astype(np.float16)
        bq = np.ascontiguousarray(
            np.concatenate(
                [b_qkv[lo:hi], b_qkv[D + lo : D + hi], b_qkv[2 * D + lo : 2 * D + hi]]
            )
        )
        wo = np.ascontiguousarray(W_out[lo:hi, :]).astype(np.float16)
        in_maps.append({"x": xf, "wqkv": wq, "bqkv": bq, "wout": wo})
    return in_maps


def kernel(x, W_qkv, b_qkv, W_out, b_out):
    b_out = np.asarray(b_out, dtype=np.float32)

    if "nc" not in _CACHE:
        _CACHE["nc"] = _build()
    nc = _CACHE["nc"]

    in_maps = _in_maps(x, W_qkv, b_qkv, W_out)
    res = run_bass_kernel_spmd(nc, in_maps, core_ids=list(range(NCORES)))
    acc = np.zeros((BT, D), dtype=np.float64)
    for c in range(NCORES):
        acc += res.results[c]["out"]
    acc += b_out
    return acc.reshape(B, T, D).astype(np.float32)
